# revision 10
# baseline (speedup 1.0000x reference)
"""Trainium2 Bass kernel for nn_MCAF (dense_transformer).

Strategy: pure data-parallel over 8 NeuronCores (batch 131072 -> 16384/core).
Heavy host-side weight folding (uniform-attention collapse, V*O fusion, LN
scale/bias folding), host-transposed feature-major input (straight contiguous
DMA loads), bf16 front-end, fp32r transformer matmuls (1 cyc/row on the PE at
N=512), fused bias+residual epilogues on the DVE (scalar_tensor_tensor),
Rsqrt-based LN, and elementwise work spread across ScalarE/DVE/GpSimd.
"""

import sys

sys.path.insert(0, "/opt/trn_rl_repo")

import numpy as np
import ml_dtypes

import concourse.bass as bass
import concourse.bacc as bacc
import concourse.tile as tile
from concourse import mybir
from concourse.bass_utils import run_bass_kernel_spmd

BF16 = ml_dtypes.bfloat16
F32 = np.float32

B_TOTAL = 131072
N_CORES = 8
B_CORE = B_TOTAL // N_CORES          # 16384
SUB = 512                            # batch columns per sub-tile
N_SUB = B_CORE // SUB                # 32
N_PAIR = N_SUB // 2                  # 16
XE_COLS = 384                        # 5 l-chunks*64 + eog 48 + pad 16
AF = mybir.ActivationFunctionType
ALU = mybir.AluOpType
dt = mybir.dt


# ---------------------------------------------------------------- host folding
def _fold_weights(w):
    """Returns (wbf [128,NBF] bf16 blob, wf32 [128,NF32] f32 blob, col maps)."""
    eeg_ow = w["eeg_ow"].astype(np.float64)
    wv = w["eeg_inw"][124:186].astype(np.float64)
    bv = w["eeg_inb"][124:186].astype(np.float64)
    Me5 = (eeg_ow @ wv) / 5.0                        # [62,62]
    c_e = eeg_ow @ bv + w["eeg_ob"].astype(np.float64)

    # --- bf16 blob ---
    NBF = 3 * 128 + 3 * 32 + 64 + 128                # obar, h, og(raw|alpha), eluW
    wbf = np.zeros((128, NBF), np.float64)
    ob_off = 0
    h_off = 3 * 128
    og_off = h_off + 3 * 32
    elu_off = og_off + 64

    # obar blocks: x320 row g=64*l+c ; M=128 cols: [obar(62) 0 0 | obar(62) 0 0]
    for t in range(3):
        rows = 64 if t == 2 else 128
        blk = np.zeros((128, 128), np.float64)
        for r in range(rows):
            g = 128 * t + r
            l, c = g // 64, g % 64
            if c < 62:
                blk[r, 0:62] = Me5[:, c]
                blk[r, 64:126] = Me5[:, c]
            elif g == 62:                            # host plants 1.0 in x320[62]
                blk[r, 0:62] = c_e
                blk[r, 64:126] = c_e
        wbf[:, ob_off + 128 * t: ob_off + 128 * (t + 1)] = blk

    # h blocks: contraction over y (same row layout), M=32
    cw = w["eeg_cw"].astype(np.float64)              # [32,62,5]
    for t in range(3):
        rows = 64 if t == 2 else 128
        blk = np.zeros((128, 32), np.float64)
        for r in range(rows):
            g = 128 * t + r
            l, c = g // 64, g % 64
            if c < 62:
                blk[r, :] = cw[:, c, l]
        wbf[:, h_off + 32 * t: h_off + 32 * (t + 1)] = blk

    # og block [128, 64]: cols 0:32 raw, 32:64 alpha/beta (xo on rows 64:112
    # of the t2 tile; host plants 1.0 in xo[33] -> row 97)
    alpha33 = float(w["eog_inw"][2, 0]) * float(w["eog_ow"][0, 0]) / 33.0
    beta = float(w["eog_inb"][2]) * float(w["eog_ow"][0, 0]) + float(w["eog_ob"][0])
    cwo = w["eog_cw"].astype(np.float64)             # [32,1,33]
    og_blk = np.zeros((128, 64), np.float64)
    for l in range(33):
        og_blk[64 + l, 0:32] = cwo[:, 0, l]
        og_blk[64 + l, 32:64] = alpha33
    og_blk[64 + 33, 32:64] = beta
    wbf[:, og_off: og_off + 64] = og_blk

    # fused (ef,of)->feat weights, elu stack rows: [eeg32, og32] per half
    fw = w["fus_w"].astype(np.float64)               # [64,128]
    W_e = fw[:, :64] @ w["eeg_fw"].astype(np.float64)    # [64,32]
    W_o = fw[:, 64:] @ w["eog_fw"].astype(np.float64)    # [64,32]
    elu_blk = np.zeros((128, 128), np.float64)
    elu_blk[0:32, 0:64] = W_e.T
    elu_blk[32:64, 0:64] = W_o.T
    elu_blk[64:96, 64:128] = W_e.T
    elu_blk[96:128, 64:128] = W_o.T
    wbf[:, elu_off: elu_off + 128] = elu_blk

    # --- f32 blob (device-side dtype float32r: full-rate PE at N=512) ---
    cols = {}
    blocks = []

    def add(name, arr):                              # arr [128, n]
        cols[name] = sum(b.shape[1] for b in blocks)
        blocks.append(arr)

    def bdiag(blk):                                  # block-diag [128,128]
        out = np.zeros((128, 128))
        out[0:64, 0:64] = blk
        out[64:128, 64:128] = blk
        return out

    add("I128", np.eye(128))
    C = np.eye(64) - 1.0 / 64.0
    add("center", bdiag(C))
    add("ones64", bdiag(np.ones((64, 64))))

    pe0 = (np.arange(64) % 2).astype(np.float64)
    b_feat = (fw[:, :64] @ w["eeg_fb"].astype(np.float64)
              + fw[:, 64:] @ w["eog_fb"].astype(np.float64)
              + w["fus_b"].astype(np.float64) + pe0
              - W_e.sum(axis=1) - W_o.sum(axis=1))   # fold elu's (e'-1)

    lay = []
    for i in range(2):
        s1 = w["tl_ln1_s"][i].astype(np.float64)
        b1v = w["tl_ln1_b"][i].astype(np.float64)
        Wvo = w["tl_ow"][i].astype(np.float64) @ w["tl_inw"][i, 128:192].astype(np.float64)
        bvo = (w["tl_ow"][i].astype(np.float64) @ w["tl_inb"][i, 128:192].astype(np.float64)
               + w["tl_ob"][i].astype(np.float64))
        Wvo_s = Wvo * s1[None, :]
        bvo_t = Wvo @ b1v + bvo
        s2 = w["tl_ln2_s"][i].astype(np.float64)
        b2v = w["tl_ln2_b"][i].astype(np.float64)
        W1 = w["tl_w1"][i].astype(np.float64)        # [256,64]
        W1_s = W1 * s2[None, :]
        b1g = W1 @ b2v + w["tl_b1"][i].astype(np.float64)   # [256]
        W2 = w["tl_w2"][i].astype(np.float64)        # [64,256]
        b2c = w["tl_b2"][i].astype(np.float64)
        lay.append((Wvo_s, bvo_t, W1_s, b1g, W2, b2c))
        add(f"attn{i}", bdiag(Wvo_s.T))
        # MLP chunk-pair packing: tile c holds hidden[64c:64c+64] for BOTH
        # halves (rows 0:64 lo, 64:128 hi) -> all matmul dsts at partition 0
        # (float32r ISA requirement) and full K=128 utilization.
        for c in range(4):
            add(f"mlp1_{i}_{c}", bdiag(W1_s[64 * c:64 * c + 64].T))
            add(f"mlp2_{i}_{c}", bdiag(W2[:, 64 * c:64 * c + 64].T))

    fn_s = w["fn_s"].astype(np.float64)
    fn_b = w["fn_b"].astype(np.float64)
    cls_w = w["cls_w"].astype(np.float64)
    cls_s = cls_w * fn_s[None, :]                    # [3,64]
    b_cls = cls_w @ fn_b + w["cls_b"].astype(np.float64)
    csT = cls_s.T                                    # [64,3]
    clsblk = np.zeros((128, 6))
    clsblk[0:64, 0:3] = csT
    clsblk[64:128, 3:6] = csT
    add("cls6", clsblk)

    # per-partition bias columns
    def col(vals128):
        return np.asarray(vals128, np.float64).reshape(128, 1)

    cb_e = w["eeg_cb"].astype(np.float64)
    cb_o = w["eog_cb"].astype(np.float64)
    add("cbcol", col(np.concatenate([cb_e, cb_o, cb_e, cb_o])))
    add("bfeat", col(np.concatenate([b_feat, b_feat])))
    for i in range(2):
        add(f"bvo{i}", col(np.concatenate([lay[i][1], lay[i][1]])))
        for c in range(4):
            g = lay[i][3][64 * c:64 * c + 64]
            add(f"b1g{i}{c}", col(np.concatenate([g, g])))
        add(f"b2c{i}", col(np.concatenate([lay[i][5], lay[i][5]])))
    bc = np.zeros(128)
    bc[0:3] = b_cls
    bc[3:6] = b_cls
    add("bcls6", col(bc))
    add("eps", col(np.full(128, 1e-5)))
    add("zero", col(np.zeros(128)))

    wf32 = np.concatenate(blocks, axis=1)
    off = {"ob": ob_off, "h": h_off, "og": og_off, "elu": elu_off}
    return wbf.astype(BF16), wf32.astype(F32), cols, off


# ---------------------------------------------------------------- device build
_CACHE = {}


def _build(nbf, nf32, cols, off):
    nc = bacc.Bacc("TRN2", target_bir_lowering=False, debug=False)
    xe_d = nc.dram_tensor("xe", [XE_COLS, B_CORE], dt.bfloat16, kind="ExternalInput")
    wbf_d = nc.dram_tensor("wbf", [128, nbf], dt.bfloat16, kind="ExternalInput")
    wf_d = nc.dram_tensor("wf32", [128, nf32], dt.float32r, kind="ExternalInput")
    y_d = nc.dram_tensor("y_fm", [6, N_PAIR * SUB], dt.float32, kind="ExternalOutput")

    # persistent sbuf arrays
    wbf_sb = nc.alloc_sbuf_tensor("wbf_sb", [128, nbf], dt.bfloat16).ap()
    wf_sb = nc.alloc_sbuf_tensor("wf_sb", [128, nf32], dt.float32r).ap()
    featA = nc.alloc_sbuf_tensor("featA", [128, N_PAIR * SUB], dt.float32r).ap()
    featB = nc.alloc_sbuf_tensor("featB", [128, N_PAIR * SUB], dt.float32r).ap()
    xn_sb = nc.alloc_sbuf_tensor("xn_sb", [128, N_PAIR * SUB], dt.float32r).ap()
    out_fm = nc.alloc_sbuf_tensor("out_fm", [8, N_PAIR * SUB], dt.float32).ap()

    def W(name, n=128):
        c0 = cols[name]
        return wf_sb[:, c0:c0 + n]

    def Wc(name, r0=0, r1=128):                      # bias columns (plain f32)
        return wf_sb[r0:r1, cols[name]:cols[name] + 1].bitcast(dt.float32)

    W2C = 2 * SUB                                    # 1024: two pairs / window
    NW = N_PAIR // 2                                 # 8 iteration windows

    with tile.TileContext(nc) as tc:
        nc.sync.dma_start(wbf_sb, wbf_d.ap())
        nc.sync.dma_start(wf_sb, wf_d.ap())

        # ====== PASS 1: front end (ACT set: exp). Two pairs per window; all
        # elementwise ops at [.,1024]; PSUM = 4x [128,1024] tensors (8 banks).
        with tc.tile_pool(name="p1sb", bufs=2) as sb, \
             tc.tile_pool(name="p1ps", bufs=1, space="PSUM") as psA:
            for pp in range(NW):
                psH = psA.tile([128, W2C], dt.float32, tag="H")
                psRO = psA.tile([128, W2C], dt.float32, tag="ro")
                for q in range(2):                   # pair q of this window
                    p = 2 * pp + q
                    t0 = sb.tile([128, W2C], dt.bfloat16, tag="t0")
                    t1 = sb.tile([128, W2C], dt.bfloat16, tag="t1")
                    t2 = sb.tile([128, W2C], dt.bfloat16, tag="t2")
                    for t, tt in enumerate((t0, t1, t2)):
                        nc.sync.dma_start(
                            tt, xe_d.ap()[128 * t:128 * (t + 1),
                                          p * W2C:(p + 1) * W2C])
                    # obar per sub-tile (incl c_e bias via data ones-row)
                    psOB = psA.tile([128, W2C], dt.float32, tag="ob")
                    for h in range(2):
                        c0 = SUB * h
                        for t, tt in enumerate((t0, t1, t2)):
                            k = 64 if t == 2 else 128
                            nc.tensor.matmul(
                                psOB[:, c0:c0 + SUB],
                                wbf_sb[0:k, off["ob"] + 128 * t: off["ob"] + 128 * t + 128],
                                tt[0:k, c0:c0 + SUB], start=(t == 0), stop=(t == 2))
                    obar2 = sb.tile([128, W2C], dt.bfloat16, tag="obar")
                    nc.vector.tensor_copy(out=obar2, in_=psOB)
                    # y = x * obar  (bf16, GpSimd: SBUF-only engine)
                    y0 = sb.tile([128, W2C], dt.bfloat16, tag="y0")
                    y1 = sb.tile([128, W2C], dt.bfloat16, tag="y1")
                    y2 = sb.tile([128, W2C], dt.bfloat16, tag="y2")
                    nc.gpsimd.tensor_mul(out=y0, in0=t0, in1=obar2)
                    nc.gpsimd.tensor_mul(out=y1, in0=t1, in1=obar2)
                    nc.gpsimd.tensor_mul(out=y2[0:64], in0=t2[0:64], in1=obar2[0:64])
                    for h in range(2):
                        c0 = SUB * h
                        o = 64 * h
                        # h (eeg) -> psH rows [0:32]/[64:96], pair col block q
                        for t, yy in enumerate((y0, y1, y2)):
                            k = 64 if t == 2 else 128
                            nc.tensor.matmul(
                                psH[o:o + 32, q * SUB:(q + 1) * SUB],
                                wbf_sb[0:k, off["h"] + 32 * t: off["h"] + 32 * t + 32],
                                yy[0:k, c0:c0 + SUB], start=(t == 0), stop=(t == 2))
                        # eog -> psRO [raw(32)|alpha(32)] rows 64h, pair col q
                        nc.tensor.matmul(
                            psRO[64 * h:64 * h + 64, q * SUB:(q + 1) * SUB],
                            wbf_sb[64:112, off["og"]:off["og"] + 64],
                            t2[64:112, c0:c0 + SUB], tile_position=(64, 64 * h))
                # og = raw * (alpha-sum), both pairs at once
                o2sb = sb.tile([128, W2C], dt.float32, tag="o2sb")
                nc.scalar.activation(o2sb[32:64], psRO[32:64], AF.Identity)
                nc.scalar.activation(o2sb[96:128], psRO[96:128], AF.Identity)
                nc.vector.tensor_tensor(out=psH[32:64], in0=psRO[0:32],
                                        in1=o2sb[32:64], op=ALU.mult)
                nc.vector.tensor_tensor(out=psH[96:128], in0=psRO[64:96],
                                        in1=o2sb[96:128], op=ALU.mult)
                # elu on stacked [eeg_lo, og_lo, eeg_hi, og_hi]
                r1 = sb.tile([128, W2C], dt.bfloat16, tag="r1")
                sm = sb.tile([128, W2C], dt.bfloat16, tag="sm")
                e1 = sb.tile([128, W2C], dt.bfloat16, tag="e1")
                eluT = sb.tile([128, W2C], dt.bfloat16, tag="elu")
                nc.scalar.activation(r1, psH, AF.Relu, bias=Wc("cbcol"))
                nc.vector.tensor_scalar(out=sm, in0=psH, scalar1=Wc("cbcol"),
                                        scalar2=0.0, op0=ALU.add, op1=ALU.min)
                nc.scalar.activation(e1, sm, AF.Exp)
                nc.gpsimd.tensor_add(out=eluT, in0=r1, in1=e1)
                # feat = W_elu.T @ elu (+b_feat), block-diag over halves
                psF = psA.tile([128, W2C], dt.float32, tag="F")
                for q in range(2):
                    nc.tensor.matmul(psF[:, q * SUB:(q + 1) * SUB],
                                     wbf_sb[:, off["elu"]:off["elu"] + 128],
                                     eluT[:, q * SUB:(q + 1) * SUB])
                nc.vector.tensor_scalar_add(featA[:, pp * W2C:(pp + 1) * W2C],
                                            psF, Wc("bfeat"))

        # =================== PASS 2: transformer ===================
        def ln_norm(ps, sb, src_ap, xn_out_ap):
            """src [128,1024] sbuf fp32r -> normalized xn. center matmuls +
            Square + ones-matmuls + Sqrt + DVE reciprocal + mul."""
            psXC = ps.tile([128, W2C], dt.float32, tag="xc")
            for q in range(2):
                cs = slice(q * SUB, (q + 1) * SUB)
                nc.tensor.matmul(psXC[:, cs], W("center"), src_ap[:, cs])
            sq = sb.tile([128, W2C], dt.float32r, tag="sq")
            nc.scalar.activation(sq, psXC, AF.Square)
            psV = ps.tile([128, W2C], dt.float32, tag="v", bufs=1)
            for q in range(2):
                cs = slice(q * SUB, (q + 1) * SUB)
                nc.tensor.matmul(psV[:, cs], W("ones64"), sq[:, cs])
            sdev = sb.tile([128, W2C], dt.float32, tag="sd")
            nc.scalar.activation(sdev, psV, AF.Sqrt, bias=Wc("eps"), scale=1.0 / 64.0)
            rstd = sb.tile([128, W2C], dt.float32, tag="rs")
            nc.vector.reciprocal_approx_fast(rstd, sdev)
            nc.vector.tensor_tensor(out=xn_out_ap, in0=psXC, in1=rstd, op=ALU.mult)

        fsrc, fdst = featA, featB
        for i in range(2):
            # ---- wave 1: LN1 + attn(+residual via I128) + LN2 (ACT: sqrt) ----
            tc.no_sync_barrier()
            with tc.tile_pool(name=f"w1s{i}", bufs=2) as sb, \
                 tc.tile_pool(name=f"w1c{i}", bufs=1, space="PSUM") as psc, \
                 tc.tile_pool(name=f"w1p{i}", bufs=2, space="PSUM") as ps:
                for pp in range(NW):
                    sl = slice(pp * W2C, (pp + 1) * W2C)
                    xn1 = sb.tile([128, W2C], dt.float32r, tag="xn1")
                    ln_norm(ps, sb, fsrc[:, sl], xn1)
                    psF2 = psc.tile([128, W2C], dt.float32, tag="c")
                    for q in range(2):
                        cs = slice(pp * W2C + q * SUB, pp * W2C + (q + 1) * SUB)
                        qs = slice(q * SUB, (q + 1) * SUB)
                        nc.tensor.matmul(psF2[:, qs], W("I128"), fsrc[:, cs],
                                         start=True, stop=False)
                        nc.tensor.matmul(psF2[:, qs], W(f"attn{i}"), xn1[:, qs],
                                         start=False, stop=True)
                    # fdst = psF2 + bvo  (residual came via I128)
                    nc.scalar.activation(fdst[:, sl], psF2, AF.Identity,
                                         bias=Wc(f"bvo{i}"))
                    ln_norm(ps, sb, fdst[:, sl], xn_sb[:, sl])
            # ---- wave 2: MLP (ACT set: gelu) ----
            tc.no_sync_barrier()
            with tc.tile_pool(name=f"w2s{i}", bufs=2) as sb, \
                 tc.tile_pool(name=f"w2c{i}", bufs=2, space="PSUM") as psc, \
                 tc.tile_pool(name=f"w2p{i}", bufs=2, space="PSUM") as ps:
                for pp in range(NW):
                    sl = slice(pp * W2C, (pp + 1) * W2C)
                    gsb = []
                    for c in range(4):               # hidden chunk c, both halves
                        gp = ps.tile([128, W2C], dt.float32, tag="g")
                        for q in range(2):
                            cs = slice(pp * W2C + q * SUB, pp * W2C + (q + 1) * SUB)
                            qs = slice(q * SUB, (q + 1) * SUB)
                            nc.tensor.matmul(gp[:, qs], W(f"mlp1_{i}_{c}"),
                                             xn_sb[:, cs])
                        g = sb.tile([128, W2C], dt.float32r, tag=f"g{c}")
                        nc.scalar.activation(g, gp, AF.Gelu, bias=Wc(f"b1g{i}{c}"))
                        gsb.append(g)
                    psF3 = psc.tile([128, W2C], dt.float32, tag="c")
                    for q in range(2):
                        qs = slice(q * SUB, (q + 1) * SUB)
                        for c in range(4):
                            nc.tensor.matmul(psF3[:, qs], W(f"mlp2_{i}_{c}"),
                                             gsb[c][:, qs],
                                             start=(c == 0), stop=(c == 3))
                    # fsrc' = (psF3 + b2c) + fdst
                    nc.vector.scalar_tensor_tensor(
                        out=fsrc[:, sl], in0=psF3, scalar=Wc(f"b2c{i}"),
                        in1=fdst[:, sl], op0=ALU.add, op1=ALU.add)
            # after layer: result lives in fsrc again (A -> B -> A)

        # ---- wave 3: final LN + classifier (ACT set: sqrt) ----
        tc.no_sync_barrier()
        with tc.tile_pool(name="w3s", bufs=2) as sb, \
             tc.tile_pool(name="w3c", bufs=1, space="PSUM") as psc, \
             tc.tile_pool(name="w3p", bufs=2, space="PSUM") as ps:
            for pp in range(NW):
                sl = slice(pp * W2C, (pp + 1) * W2C)
                xn3 = sb.tile([128, W2C], dt.float32r, tag="xn3")
                ln_norm(ps, sb, fsrc[:, sl], xn3)
                psO = psc.tile([128, W2C], dt.float32, tag="c")
                for q in range(2):
                    qs = slice(q * SUB, (q + 1) * SUB)
                    nc.tensor.matmul(psO[0:6, qs], W("cls6", n=6), xn3[:, qs])
                nc.vector.tensor_scalar_add(out_fm[0:6, sl], psO[0:6],
                                            Wc("bcls6", 0, 6))
        nc.sync.dma_start(y_d.ap(), out_fm[0:6, :])

    nc.compile()
    return nc


# ---------------------------------------------------------------- entry point
def kernel(**inputs):
    w = {k: np.asarray(v) for k, v in inputs.items()}
    wbf, wf32, cols, off = _fold_weights(w)

    # x320 l-major: xeT[64*l + c, :] = eeg[:, c, l]; row 62 = 1.0 (c_e bias row)
    # Feature-major host layout -> straight contiguous DMA loads on-device.
    eeg = w["eeg"].astype(F32)
    xeT = np.zeros((XE_COLS, B_TOTAL), F32)
    xeT[0:320].reshape(5, 64, B_TOTAL)[:, 0:62, :] = eeg.transpose(2, 1, 0)
    xeT[62] = 1.0
    xeT[320:353] = w["eog"].astype(F32)[:, 0, :].T
    xeT[353] = 1.0                                   # beta bias row (xo row 33)
    xeT = xeT.astype(BF16)

    key = ("prog", wbf.shape[1], wf32.shape[1])
    if key not in _CACHE:
        _CACHE[key] = _build(wbf.shape[1], wf32.shape[1], cols, off)
    nc = _CACHE[key]

    in_maps = []
    for k in range(N_CORES):
        in_maps.append({
            "xe": np.ascontiguousarray(xeT[:, k * B_CORE:(k + 1) * B_CORE]),
            "wbf": wbf, "wf32": wf32,
        })
    res = run_bass_kernel_spmd(nc, in_maps, core_ids=list(range(N_CORES)))

    out = np.empty((B_TOTAL, 3), F32)
    for k in range(N_CORES):
        y = res.results[k]["y_fm"].reshape(2, 3, N_PAIR, SUB)
        out[k * B_CORE:(k + 1) * B_CORE] = (
            y.transpose(2, 0, 3, 1).reshape(B_CORE, 3))
    return out


if __name__ == "__main__":
    import reference
    ins = {k: np.asarray(v) for k, v in reference.setup_inputs().items()}
    got = kernel(**ins)
    exp = np.asarray(reference.reference(**ins))
    err = np.abs(got - exp).max() / (np.abs(exp).max() + 1e-9)
    print("Relative error:", err)
